# revision 18
# baseline (speedup 1.0000x reference)
"""Deformable 2D feature aggregator — Trainium2 Bass kernel, 8-core SPMD. v3.

Problem: B=2, C=128, H=96, W=160, P=9 points, G=8 groups.
  value = conv1x1(feats); w = softmax over P of conv1x1(feats); offs = conv1x1(feats)
  pts = anchors + offs; out_proj(conv-weighted bilinear gather of value at pts).

Sharding: 8 cores = 2 batches x 4 query-slices; each core builds the full
(rotated) value map for its batch, then gathers per-query corner windows.

v3 vs v2:
  - coords -> idxf -> idx16 chain emitted FIRST so the gather stream (gated on
    idx16 + valscr2) starts ~50us earlier; softmax/kw overlap under gathers.
  - last two gather chunks are single-tile so the final combine tail is half.
  - gather pool triple-buffered to absorb combine latency between chunks.
"""
import sys

sys.path.insert(0, "/opt/trn_rl_repo")

import numpy as np
import ml_dtypes

import concourse.bass as bass
import concourse.bacc as bacc
import concourse.mybir as mybir
import concourse.tile as tile
from concourse import library_config
from concourse.ap import AP

# problem constants (hardcoded per harness contract)
B, C, H, W = 2, 128, 96, 160
HW = H * W                     # 15360
P, G, GC = 9, 8, 16
NCORES = 8
QS = B * HW // NCORES          # 3840 queries per core
NT = QS // 128                 # 30 query tiles
TCH = 2                        # query tiles per gather chunk
NCH = NT // TCH                # 15 gather chunks
NP_ = NT * P                   # 270 points per partition-row
NIDX_CH = TCH * 128 * P        # 2304 gather indices per chunk
SHIFT = 1024.0                 # floor-bias (exact in f32 for our range)
NPXT = HW // 128               # 120 pixel tiles
VT = 12                        # pixel tiles per valscr2 write chunk
NCHV = NPXT // VT              # 24 write chunks

f32 = mybir.dt.float32
bf16 = mybir.dt.bfloat16
i16 = mybir.dt.int16
Alu = mybir.AluOpType
Act = mybir.ActivationFunctionType
Ax = mybir.AxisListType

_CACHE: dict = {}


def _build_nc():
    nc = bacc.Bacc()

    feats16 = nc.dram_tensor("feats16", [C, HW], bf16, kind="ExternalInput")
    feats32 = nc.dram_tensor("feats32", [C, QS], f32, kind="ExternalInput")
    anch = nc.dram_tensor("anch", [128, NT * 2], f32, kind="ExternalInput")
    vwT16 = nc.dram_tensor("vwT16", [C, C], bf16, kind="ExternalInput")
    w72T = nc.dram_tensor("w72T", [C, 72], bf16, kind="ExternalInput")
    w18T = nc.dram_tensor("w18T", [C, 18], f32, kind="ExternalInput")
    owT16 = nc.dram_tensor("owT16", [C, C], bf16, kind="ExternalInput")
    b72r = nc.dram_tensor("b72r", [128, 72], f32, kind="ExternalInput")
    b18r = nc.dram_tensor("b18r", [128, 18], f32, kind="ExternalInput")
    bvr = nc.dram_tensor("bvr", [128, C], f32, kind="ExternalInput")
    outb = nc.dram_tensor("outb", [128, 1], f32, kind="ExternalInput")
    oneh = nc.dram_tensor("oneh", [128, 8 * 128], f32, kind="ExternalInput")
    ident16 = nc.dram_tensor("ident16", [128, 128], bf16, kind="ExternalInput")
    rotoff = nc.dram_tensor("rotoff", [128, 1], f32, kind="ExternalInput")
    out_d = nc.dram_tensor("out", [C, QS], f32, kind="ExternalOutput")

    with tile.TileContext(nc) as tc, nc.allow_low_precision("bf16 combine by design"):
        with (
            tc.tile_pool(name="const", bufs=1) as cpool,
            tc.tile_pool(name="stage", bufs=1) as spool,
            tc.tile_pool(name="vsb", bufs=3) as vpool,
            tc.tile_pool(name="g", bufs=2) as gpool,
            tc.tile_pool(name="tprime", bufs=2) as tpool,
            tc.tile_pool(name="aggp", bufs=2) as apool,
            tc.tile_pool(name="ps", bufs=1, space="PSUM") as pspool,
            tc.tile_pool(name="dram", bufs=1, space="DRAM") as dpool,
        ):
            # ---- persistent loads (query-path tensors first: they gate idx16) ----
            f32s = spool.tile([128, QS], f32)
            nc.sync.dma_start(f32s[:], feats32[:])
            w72T_s = cpool.tile([C, 72], bf16)
            nc.sync.dma_start(w72T_s[:], w72T[:])
            w18T_s = cpool.tile([C, 18], f32)
            nc.sync.dma_start(w18T_s[:], w18T[:])
            anch_s = cpool.tile([128, NT * 2], f32)
            nc.sync.dma_start(anch_s[:], anch[:])
            rot_s = cpool.tile([128, 1], f32)
            nc.sync.dma_start(rot_s[:], rotoff[:])
            oneh_s = cpool.tile([128, 8 * 128], f32)
            nc.sync.dma_start(oneh_s[:], oneh[:])
            b72_s = cpool.tile([128, 72], f32)
            nc.sync.dma_start(b72_s[:], b72r[:])
            b18_s = cpool.tile([128, 18], f32)
            nc.sync.dma_start(b18_s[:], b18r[:])
            f16s = spool.tile([128, HW], bf16)
            nc.sync.dma_start(f16s[:], feats16[:])
            vwT_s = cpool.tile([C, C], bf16)
            nc.sync.dma_start(vwT_s[:], vwT16[:])
            owT_s = cpool.tile([C, C], bf16)
            nc.sync.dma_start(owT_s[:], owT16[:])
            bvr_s = cpool.tile([128, C], f32)
            nc.sync.dma_start(bvr_s[:], bvr[:])
            outb_s = cpool.tile([128, 1], f32)
            nc.sync.dma_start(outb_s[:], outb[:])
            ident_s = cpool.tile([128, 128], bf16)
            nc.sync.dma_start(ident_s[:], ident16[:])

            # dummy gather: preloads the GPSIMD gather-library IRAM during
            # the head so the first real gather doesn't pay the ~12us load.
            dumscr = dpool.tile([17, 2 * C], bf16)
            dumsrc = AP(tensor=dumscr.tensor, offset=dumscr[:, :].offset,
                        ap=[[2 * C, 16], [1, 4 * C]])
            dumidx = spool.tile([128, 1], i16)
            nc.vector.memset(dumidx[:], 0)
            dumout = spool.tile([128, 1, 4 * C], bf16)
            nc.gpsimd.dma_gather(
                dumout[:, :, :], dumsrc, dumidx[:, 0:1],
                num_idxs=16, num_idxs_reg=16,
                elem_size=4 * C, elem_step=2 * C, single_packet=False,
            )

            # pair-row scratch: record r = [V_rot(r), V_rot((r+W) mod HW)].
            # +1 pad record (= record 0) backs the r0+1 read at r0 = HW-1,
            # which is reachable after rotation.
            valscr2 = dpool.tile([HW + 1, 2 * C], bf16)

            proj72 = spool.tile([128, NT * 72], bf16)
            proj18 = spool.tile([128, NT * 18], f32)

            def tt(out, in0, in1, op):
                nc.vector.tensor_tensor(out=out, in0=in0, in1=in1, op=op)

            # ---- phase A0: query projections (offsets f32; wlog bf16) ----
            for t in range(NT):
                pp18 = pspool.tile([128, 18], f32, tag="pp", bufs=3,
                                   name=f"pp18_{t}")
                nc.tensor.matmul(pp18[:], f32s[:, t * 128 : (t + 1) * 128],
                                 w18T_s[:], start=True, stop=True)
                nc.vector.tensor_tensor(
                    out=proj18[:, t * 18 : (t + 1) * 18],
                    in0=pp18[:], in1=b18_s[:], op=Alu.add)
            for t in range(NT):
                pp72 = pspool.tile([128, 72], f32, tag="pp", bufs=3,
                                   name=f"pp72_{t}")
                nc.tensor.matmul(pp72[:], f16s[:, t * 128 : (t + 1) * 128],
                                 w72T_s[:], start=True, stop=True)
                nc.vector.tensor_tensor(
                    out=proj72[:, t * 72 : (t + 1) * 72],
                    in0=pp72[:], in1=b72_s[:], op=Alu.add)

            # ---- phase B0: coords -> gather record indices (critical path) ----
            px = spool.tile([128, NP_], f32, tag="px")
            py = spool.tile([128, NP_], f32, tag="py")
            offs_x = AP(tensor=proj18.tensor, offset=proj18[:, :].offset,
                        ap=[[proj18[:, :].ap[0][0], 128], [18, NT], [2, P]])
            offs_y = AP(tensor=proj18.tensor, offset=proj18[:, :].offset + 1,
                        ap=[[proj18[:, :].ap[0][0], 128], [18, NT], [2, P]])
            anx = AP(tensor=anch_s.tensor, offset=anch_s[:, :].offset,
                     ap=[[anch_s[:, :].ap[0][0], 128], [2, NT], [0, P]])
            any_ = AP(tensor=anch_s.tensor, offset=anch_s[:, :].offset + 1,
                      ap=[[anch_s[:, :].ap[0][0], 128], [2, NT], [0, P]])
            tt(px[:, :].rearrange("p (t q) -> p t q", q=P), offs_x, anx, Alu.add)
            tt(py[:, :].rearrange("p (t q) -> p t q", q=P), offs_y, any_, Alu.add)

            xp = spool.tile([128, NP_], f32)
            yp = spool.tile([128, NP_], f32)
            nc.scalar.activation(xp[:], px[:], Act.Copy, bias=SHIFT - 0.5, scale=float(W))
            nc.scalar.activation(yp[:], py[:], Act.Copy, bias=SHIFT - 0.5, scale=float(H))
            # floor via round(x-0.5): (x + (2^23-0.5)) - 2^23. At integer x the
            # half-even tie may floor one low with frac 1.0 — an equivalent
            # bilinear weighting, so interpolation is unchanged.
            MAGIC = float(1 << 23)
            xf = spool.tile([128, NP_], f32, tag="px")   # reuse px slot
            yf = spool.tile([128, NP_], f32, tag="py")   # reuse py slot
            nc.vector.tensor_scalar(out=xf[:], in0=xp[:], scalar1=MAGIC - 0.5,
                                    scalar2=MAGIC, op0=Alu.add, op1=Alu.subtract)
            nc.vector.tensor_scalar(out=yf[:], in0=yp[:], scalar1=MAGIC - 0.5,
                                    scalar2=MAGIC, op0=Alu.add, op1=Alu.subtract)

            xg = spool.tile([128, NP_], f32)
            nc.vector.tensor_scalar(out=xg[:], in0=xf[:], scalar1=SHIFT, scalar2=0.0,
                                    op0=Alu.subtract, op1=Alu.max)
            nc.vector.tensor_scalar(out=xg[:], in0=xg[:], scalar1=float(W - 2),
                                    scalar2=None, op0=Alu.min)
            yg = spool.tile([128, NP_], f32)
            nc.vector.tensor_scalar(out=yg[:], in0=yf[:], scalar1=SHIFT, scalar2=0.0,
                                    op0=Alu.subtract, op1=Alu.max)
            nc.vector.tensor_scalar(out=yg[:], in0=yg[:], scalar1=float(H - 2),
                                    scalar2=None, op0=Alu.min)

            # idx = (yg*W + xg - rotoff) mod HW
            v1 = spool.tile([128, NP_], f32)
            v2 = spool.tile([128, NP_], f32)
            idxf = spool.tile([128, NP_], f32)
            nc.scalar.activation(v1[:], yg[:], Act.Copy, bias=0.0, scale=float(W))
            tt(idxf[:], v1[:], xg[:], Alu.add)
            nc.vector.tensor_scalar(out=idxf[:], in0=idxf[:], scalar1=rot_s[:, 0:1],
                                    scalar2=None, op0=Alu.subtract)
            nc.vector.tensor_scalar(out=v2[:], in0=idxf[:], scalar1=0.0,
                                    scalar2=float(HW), op0=Alu.is_lt, op1=Alu.mult)
            tt(idxf[:], idxf[:], v2[:], Alu.add)

            # ---- phase A1: bf16 value map over the whole rotated image ----
            def emit_value_chunk(v):
                vsb5 = vpool.tile([128, VT * C], bf16, tag="vsb5", bufs=2,
                                  name=f"vsb5_{v}")
                for k4 in range(VT // 4):
                    t = v * VT + 4 * k4
                    vps = pspool.tile([128, 4 * C], f32, tag="vps", bufs=3,
                                      name=f"vps{t}")
                    for j in range(4):
                        nc.tensor.matmul(
                            vps[:, j * C : (j + 1) * C],
                            f16s[:, (t + j) * 128 : (t + j + 1) * 128],
                            vwT_s[:], start=True, stop=True)
                    nc.scalar.copy(vsb5[:, 4 * k4 * C : (4 * k4 + 4) * C], vps[:])
                base = v * VT * 128  # first pixel (row) of this chunk
                # first half: rows [base, base+640), cols 0:C
                dst1 = AP(tensor=valscr2.tensor,
                          offset=valscr2[:, :].offset + base * 2 * C,
                          ap=[[2 * C, 128], [128 * 2 * C, VT], [1, C]])
                src1 = AP(tensor=vsb5.tensor, offset=vsb5[:, :].offset,
                          ap=[[vsb5[:, :].ap[0][0], 128], [C, VT], [1, C]])
                nc.sync.dma_start(dst1, src1)
                # second half: rows [(base - W) mod HW ...), cols C:2C
                lo = base - W
                if lo >= 0:
                    dst2 = AP(tensor=valscr2.tensor,
                              offset=valscr2[:, :].offset + lo * 2 * C + C,
                              ap=[[2 * C, 128], [128 * 2 * C, VT], [1, C]])
                    nc.sync.dma_start(dst2, src1)
                else:
                    # v == 0: rows [HW-160, HW) from (b=0 all p) + (b=1 p<32),
                    # then rows [0, 96) from (b=1 p>=32), rows [96, 480) b=2..4
                    d_a = AP(tensor=valscr2.tensor,
                             offset=valscr2[:, :].offset + (HW - W) * 2 * C + C,
                             ap=[[2 * C, 128], [1, C]])
                    s_a = AP(tensor=vsb5.tensor, offset=vsb5[:, :].offset,
                             ap=[[vsb5[:, :].ap[0][0], 128], [1, C]])
                    nc.sync.dma_start(d_a, s_a)
                    d_b = AP(tensor=valscr2.tensor,
                             offset=valscr2[:, :].offset + (HW - 32) * 2 * C + C,
                             ap=[[2 * C, 32], [1, C]])
                    s_b = AP(tensor=vsb5.tensor, offset=vsb5[:, :].offset + C,
                             ap=[[vsb5[:, :].ap[0][0], 32], [1, C]])
                    nc.sync.dma_start(d_b, s_b)
                    d_c = AP(tensor=valscr2.tensor,
                             offset=valscr2[:, :].offset + 0 * 2 * C + C,
                             ap=[[2 * C, 96], [1, C]])
                    s_c = AP(tensor=vsb5.tensor,
                             offset=vsb5[32:, :].offset + C,
                             ap=[[vsb5[:, :].ap[0][0], 96], [1, C]])
                    nc.sync.dma_start(d_c, s_c)
                    d_d = AP(tensor=valscr2.tensor,
                             offset=valscr2[:, :].offset + 96 * 2 * C + C,
                             ap=[[2 * C, 128], [128 * 2 * C, VT - 2], [1, C]])
                    s_d = AP(tensor=vsb5.tensor,
                             offset=vsb5[:, :].offset + 2 * C,
                             ap=[[vsb5[:, :].ap[0][0], 128], [C, VT - 2], [1, C]])
                    nc.sync.dma_start(d_d, s_d)
                    # pad record HW = record 0 = [V_rot(0), V_rot(W)]
                    d_p0 = AP(tensor=valscr2.tensor,
                              offset=valscr2[:, :].offset + HW * 2 * C,
                              ap=[[2 * C, 1], [1, C]])
                    s_p0 = AP(tensor=vsb5.tensor, offset=vsb5[:, :].offset,
                              ap=[[vsb5[:, :].ap[0][0], 1], [1, C]])
                    nc.sync.dma_start(d_p0, s_p0)
                    d_p1 = AP(tensor=valscr2.tensor,
                              offset=valscr2[:, :].offset + HW * 2 * C + C,
                              ap=[[2 * C, 1], [1, C]])
                    s_p1 = AP(tensor=vsb5.tensor,
                              offset=vsb5[32:, :].offset + C,
                              ap=[[vsb5[:, :].ap[0][0], 1], [1, C]])
                    nc.sync.dma_start(d_p1, s_p1)


            for v in range(3):
                emit_value_chunk(v)

            # chunk schedule: 14 double-tile chunks + 2 single-tile chunks
            # (small tail chunks shorten the final gather-drain + combine).
            CHUNKS = [(2 * i, 2) for i in range(14)] + [(28, 1), (29, 1)]
            COLOFF = []
            acc = 0
            for (_, ntile) in CHUNKS:
                COLOFF.append(acc)
                acc += ntile * P * 8

            # idx16: position i in chunk = jj*128 + q (jj = tt*P + pt),
            # stored at idx16[q%16 (+16a), coloff + 8*jj + q//16]
            idx16 = spool.tile([128, NT * P * 8], i16)
            for qh in range(8):
                i16ps = pspool.tile([128, NP_], f32, tag="pp", bufs=3,
                                    name=f"i16ps{qh}")
                nc.tensor.matmul(i16ps[:], oneh_s[:, qh * 128 : (qh + 1) * 128],
                                 idxf[:], start=True, stop=True)
                dst = AP(tensor=idx16.tensor, offset=idx16[:, :].offset + qh,
                         ap=[[idx16[:, :].ap[0][0], 128],
                             [2 * P * 8, 14], [P * 8, 2], [8, P]])
                src = AP(tensor=i16ps.tensor, offset=i16ps[:, :].offset,
                         ap=[[i16ps[:, :].ap[0][0], 128],
                             [2 * P, 14], [P, 2], [1, P]])
                nc.vector.tensor_copy(dst, src)
                # tail tiles 28, 29 (single-tile chunks, contiguous blocks)
                dst_t = AP(tensor=idx16.tensor,
                           offset=idx16[:, :].offset + qh + 28 * P * 8,
                           ap=[[idx16[:, :].ap[0][0], 128], [P * 8, 2], [8, P]])
                src_t = AP(tensor=i16ps.tensor,
                           offset=i16ps[:, :].offset + 28 * P,
                           ap=[[i16ps[:, :].ap[0][0], 128], [P, 2], [1, P]])
                nc.vector.tensor_copy(dst_t, src_t)

            for v in range(3, NCHV):
                emit_value_chunk(v)

            # ---- phase B1: softmax / bilinear weights (overlaps gathers) ----
            wx = spool.tile([128, NP_], f32)
            wy = spool.tile([128, NP_], f32)
            tt(wx[:], xp[:], xf[:], Alu.subtract)
            tt(wy[:], yp[:], yf[:], Alu.subtract)
            ux = spool.tile([128, NP_], f32)
            uy = spool.tile([128, NP_], f32)
            nc.scalar.activation(ux[:], wx[:], Act.Copy, bias=1.0, scale=-1.0)
            nc.scalar.activation(uy[:], wy[:], Act.Copy, bias=1.0, scale=-1.0)

            # softmax over points
            wmax = spool.tile([128, NT * G], f32, tag="wmax")
            wl_gp = AP(tensor=proj72.tensor, offset=proj72[:, :].offset,
                       ap=[[proj72[:, :].ap[0][0], 128], [72, NT], [1, G], [G, P]])
            nc.vector.tensor_reduce(out=wmax[:, :].rearrange("p (t g) -> p t g", g=G),
                                    in_=wl_gp, axis=Ax.X, op=Alu.max)
            smf = spool.tile([128, NT * P * G], f32, tag="smf")
            wl_pg = AP(tensor=proj72.tensor, offset=proj72[:, :].offset,
                       ap=[[proj72[:, :].ap[0][0], 128], [72, NT], [G, P], [1, G]])
            wmax_b = AP(tensor=wmax.tensor, offset=wmax[:, :].offset,
                        ap=[[wmax[:, :].ap[0][0], 128], [G, NT], [0, P], [1, G]])
            tt(smf[:, :].rearrange("p (t q g) -> p t q g", q=P, g=G),
               wl_pg, wmax_b, Alu.subtract)
            nc.scalar.activation(smf[:], smf[:], Act.Exp)
            ssum = spool.tile([128, NT * G], f32, tag="wmax")
            sm_gp = AP(tensor=smf.tensor, offset=smf[:, :].offset,
                       ap=[[smf[:, :].ap[0][0], 128], [P * G, NT], [1, G], [G, P]])
            nc.vector.tensor_reduce(out=ssum[:, :].rearrange("p (t g) -> p t g", g=G),
                                    in_=sm_gp, axis=Ax.X, op=Alu.add)
            rcps = spool.tile([128, NT * G], f32)
            nc.vector.reciprocal(rcps[:], ssum[:])
            wsm = spool.tile([128, NT * P * G], bf16)
            rcp_b = AP(tensor=rcps.tensor, offset=rcps[:, :].offset,
                       ap=[[rcps[:, :].ap[0][0], 128], [G, NT], [0, P], [1, G]])
            tt(wsm[:, :].rearrange("p (t q g) -> p t q g", q=P, g=G),
               smf[:, :].rearrange("p (t q g) -> p t q g", q=P, g=G),
               rcp_b, Alu.mult)

            # x-validity masks (with pair-clamp weight swap)
            tA = spool.tile([128, NP_], f32)
            tB = spool.tile([128, NP_], f32)
            mAx = spool.tile([128, NP_], f32)
            nc.vector.tensor_scalar(out=tA[:], in0=xf[:], scalar1=SHIFT,
                                    scalar2=None, op0=Alu.is_ge)
            nc.vector.tensor_scalar(out=tB[:], in0=xf[:], scalar1=SHIFT + W - 2,
                                    scalar2=None, op0=Alu.is_le)
            tt(mAx[:], tA[:], tB[:], Alu.mult)
            mBx = spool.tile([128, NP_], f32)
            nc.vector.tensor_scalar(out=mBx[:], in0=xf[:], scalar1=SHIFT - 1.0,
                                    scalar2=None, op0=Alu.is_equal)
            mCx = spool.tile([128, NP_], f32)
            nc.vector.tensor_scalar(out=mCx[:], in0=xf[:], scalar1=SHIFT + W - 1,
                                    scalar2=None, op0=Alu.is_equal)

            bx = spool.tile([128, NT * P * 2], f32)   # (t, pt, side)
            tt(v1[:], ux[:], mAx[:], Alu.mult)
            tt(v2[:], wx[:], mBx[:], Alu.mult)
            bx0 = AP(tensor=bx.tensor, offset=bx[:, :].offset,
                     ap=[[bx[:, :].ap[0][0], 128], [2, NP_]])
            tt(bx0, v1[:], v2[:], Alu.add)
            tt(v1[:], wx[:], mAx[:], Alu.mult)
            tt(v2[:], ux[:], mCx[:], Alu.mult)
            bx1 = AP(tensor=bx.tensor, offset=bx[:, :].offset + 1,
                     ap=[[bx[:, :].ap[0][0], 128], [2, NP_]])
            tt(bx1, v1[:], v2[:], Alu.add)

            # y masks (swap form, mirrors x): slot0 = row yg, slot1 = row yg+1
            my = spool.tile([128, NP_], f32)
            nc.vector.tensor_scalar(out=tA[:], in0=yf[:], scalar1=SHIFT,
                                    scalar2=None, op0=Alu.is_ge)
            nc.vector.tensor_scalar(out=tB[:], in0=yf[:], scalar1=SHIFT + H - 2,
                                    scalar2=None, op0=Alu.is_le)
            tt(my[:], tA[:], tB[:], Alu.mult)
            myB = spool.tile([128, NP_], f32)
            nc.vector.tensor_scalar(out=myB[:], in0=yf[:], scalar1=SHIFT - 1.0,
                                    scalar2=None, op0=Alu.is_equal)
            myC = spool.tile([128, NP_], f32)
            nc.vector.tensor_scalar(out=myC[:], in0=yf[:], scalar1=SHIFT + H - 1,
                                    scalar2=None, op0=Alu.is_equal)
            by = spool.tile([128, NT * P * 2], f32)   # (t, pt, yy)
            by0 = AP(tensor=by.tensor, offset=by[:, :].offset,
                     ap=[[by[:, :].ap[0][0], 128], [2, NP_]])
            by1 = AP(tensor=by.tensor, offset=by[:, :].offset + 1,
                     ap=[[by[:, :].ap[0][0], 128], [2, NP_]])
            tt(v1[:], uy[:], my[:], Alu.mult)
            tt(v2[:], wy[:], myB[:], Alu.mult)
            tt(by0, v1[:], v2[:], Alu.add)
            tt(v1[:], wy[:], my[:], Alu.mult)
            tt(v2[:], uy[:], myC[:], Alu.mult)
            tt(by1, v1[:], v2[:], Alu.add)

            # cw[t, pt, x, yy] = bx[t,pt,x] * by[t,pt,yy]
            cw = spool.tile([128, NT * P * 4], bf16)
            for x in range(2):
                by_v = AP(tensor=by.tensor, offset=by[:, :].offset,
                          ap=[[by[:, :].ap[0][0], 128], [2 * P, NT], [2, P], [1, 2]])
                bx_x = AP(tensor=bx.tensor, offset=bx[:, :].offset + x,
                          ap=[[bx[:, :].ap[0][0], 128], [2 * P, NT], [2, P], [0, 2]])
                cw_x = AP(tensor=cw.tensor, offset=cw[:, :].offset + 2 * x,
                          ap=[[cw[:, :].ap[0][0], 128], [4 * P, NT], [4, P], [1, 2]])
                tt(cw_x, by_v, bx_x, Alu.mult)

            # kw[t, pt, rs, g] = cw[t, pt, rs] * wsm[t, pt, g]  (bf16)
            kw = spool.tile([128, NT * P * 4 * G], bf16)
            for rs in range(4):
                cw_rs = AP(tensor=cw.tensor, offset=cw[:, :].offset + rs,
                           ap=[[cw[:, :].ap[0][0], 128], [4 * P, NT], [4, P], [0, G]])
                w_v = AP(tensor=wsm.tensor, offset=wsm[:, :].offset,
                         ap=[[wsm[:, :].ap[0][0], 128], [P * G, NT], [G, P], [1, G]])
                kw_rs = AP(tensor=kw.tensor, offset=kw[:, :].offset + rs * G,
                           ap=[[kw[:, :].ap[0][0], 128], [4 * P * G, NT], [4 * G, P], [1, G]])
                tt(kw_rs, cw_rs, w_v, Alu.mult)

            # sumcoef[t, g] = sum_pt wsm * (bx0+bx1)*(by0+by1)   (for value_b)
            bx0r = AP(tensor=bx.tensor, offset=bx[:, :].offset,
                      ap=[[bx[:, :].ap[0][0], 128], [2, NP_]])
            bx1r = AP(tensor=bx.tensor, offset=bx[:, :].offset + 1,
                      ap=[[bx[:, :].ap[0][0], 128], [2, NP_]])
            by0r = AP(tensor=by.tensor, offset=by[:, :].offset,
                      ap=[[by[:, :].ap[0][0], 128], [2, NP_]])
            by1r = AP(tensor=by.tensor, offset=by[:, :].offset + 1,
                      ap=[[by[:, :].ap[0][0], 128], [2, NP_]])
            tt(v1[:], bx0r, bx1r, Alu.add)
            tt(v2[:], by0r, by1r, Alu.add)
            bws = spool.tile([128, NP_], bf16)
            tt(bws[:], v1[:], v2[:], Alu.mult)
            wp = spool.tile([128, NT * P * G], bf16, tag="smf")  # reuse smf slot
            bws_b = AP(tensor=bws.tensor, offset=bws[:, :].offset,
                       ap=[[bws[:, :].ap[0][0], 128], [P, NT], [1, P], [0, G]])
            tt(wp[:, :].rearrange("p (t q g) -> p t q g", q=P, g=G),
               wsm[:, :].rearrange("p (t q g) -> p t q g", q=P, g=G),
               bws_b, Alu.mult)
            sumcoef = spool.tile([128, NT * G], f32)
            wp_gp = AP(tensor=wp.tensor, offset=wp[:, :].offset,
                       ap=[[wp[:, :].ap[0][0], 128], [P * G, NT], [1, G], [G, P]])
            nc.vector.tensor_reduce(out=sumcoef[:, :].rearrange("p (t g) -> p t g", g=G),
                                    in_=wp_gp, axis=Ax.X, op=Alu.add)

            # ---- phase C: gather chunks + combine ----
            val_src = AP(tensor=valscr2.tensor, offset=valscr2[:, :].offset,
                         ap=[[2 * C, HW], [1, 4 * C]])

            for ch, (t0, ntile) in enumerate(CHUNKS):
                nidx = ntile * P * 128
                gt = gpool.tile([128, TCH * P, 4 * C], bf16, tag="g", bufs=3,
                                name=f"g{ch}")
                last = ch == len(CHUNKS) - 1
                if not last:
                    nc.gpsimd.dma_gather(
                        gt[:, : ntile * P, :], val_src,
                        idx16[:, COLOFF[ch] : COLOFF[ch] + nidx // 16],
                        num_idxs=nidx, num_idxs_reg=nidx,
                        elem_size=4 * C, elem_step=2 * C, single_packet=False,
                    )
                else:
                    # split the final gather so the tail DMA drain is short
                    nA = 5 * 128
                    nc.gpsimd.dma_gather(
                        gt[:, :5, :], val_src,
                        idx16[:, COLOFF[ch] : COLOFF[ch] + nA // 16],
                        num_idxs=nA, num_idxs_reg=nA,
                        elem_size=4 * C, elem_step=2 * C, single_packet=False,
                    )
                    nB = 4 * 128
                    nc.gpsimd.dma_gather(
                        gt[:, 5:9, :], val_src,
                        idx16[:, COLOFF[ch] + nA // 16 : COLOFF[ch] + P * 8],
                        num_idxs=nB, num_idxs_reg=nB,
                        elem_size=4 * C, elem_step=2 * C, single_packet=False,
                    )

                for tt_ in range(ntile):
                    t = t0 + tt_
                    # tp[q, (j36, c)] = gt * kw  (j36 = (pt, x, yy), c = (g, gc))
                    tp = tpool.tile([128, P * 4 * C], bf16, tag="tp", bufs=1,
                                    name=f"tp{t}")
                    g_v = AP(tensor=gt.tensor,
                             offset=gt[:, :, :].offset + tt_ * P * 4 * C,
                             ap=[[gt[:, :, :].ap[0][0], 128],
                                 [C, P * 4], [GC, G], [1, GC]])
                    kw_v = AP(tensor=kw.tensor, offset=kw[:, :].offset + t * P * 4 * G,
                              ap=[[kw[:, :].ap[0][0], 128],
                                  [G, P * 4], [1, G], [0, GC]])
                    tp_v = AP(tensor=tp.tensor, offset=tp[:, :].offset,
                              ap=[[tp[:, :].ap[0][0], 128],
                                  [C, P * 4], [GC, G], [1, GC]])
                    nc.vector.tensor_tensor(out=tp_v, in0=g_v, in1=kw_v, op=Alu.mult)

                    # pre-add yy pairs (2x mode): tp2[q, ((pt,x), c)]
                    tp2 = tpool.tile([128, P * 2 * C], bf16, tag="tp2", name=f"tp2{t}")
                    in0 = AP(tensor=tp.tensor, offset=tp[:, :].offset,
                             ap=[[tp[:, :].ap[0][0], 128], [2 * C, 2 * P], [1, C]])
                    in1 = AP(tensor=tp.tensor, offset=tp[:, :].offset + C,
                             ap=[[tp[:, :].ap[0][0], 128], [2 * C, 2 * P], [1, C]])
                    out2 = AP(tensor=tp2.tensor, offset=tp2[:, :].offset,
                              ap=[[tp2[:, :].ap[0][0], 128], [C, 2 * P], [1, C]])
                    nc.vector.tensor_tensor(out=out2, in0=in0, in1=in1, op=Alu.add)

                    # ebias[q, c] = value_b[c] * sumcoef[q, g(c)]  (bf16)
                    ebias = apool.tile([128, C], bf16, tag="eb", bufs=1, name=f"eb{t}")
                    sc_v = AP(tensor=sumcoef.tensor,
                              offset=sumcoef[:, :].offset + t * G,
                              ap=[[sumcoef[:, :].ap[0][0], 128], [1, G], [0, GC]])
                    bv_v = bvr_s[:, :].rearrange("p (g c) -> p g c", g=G)
                    nc.vector.tensor_tensor(
                        out=ebias[:, :].rearrange("p (g c) -> p g c", g=G),
                        in0=sc_v, in1=bv_v, op=Alu.mult)

                    # 19 transpose-accumulate matmuls: aggT[c, q] in PSUM
                    aggT = pspool.tile([128, 128], f32, tag="mm128", bufs=2,
                                       name=f"aggT{t}")
                    for k in range(2 * P):
                        nc.tensor.matmul(aggT[:], tp2[:, k * C : (k + 1) * C],
                                         ident_s[:], start=(k == 0), stop=False)
                    nc.tensor.matmul(aggT[:], ebias[:], ident_s[:],
                                     start=False, stop=True)

                    # out-projection straight off the accumulated PSUM
                    aT = apool.tile([128, 128], bf16, tag="aT", bufs=1, name=f"aT{t}")
                    nc.scalar.copy(aT[:], aggT[:])
                    fops = pspool.tile([128, C], f32, tag="mm128", bufs=2,
                                       name=f"fo{t}")
                    nc.tensor.matmul(fops[:], owT_s[:], aT[:], start=True, stop=True)
                    fo_sb = apool.tile([128, C], f32, tag="fosb", name=f"fosb{t}")
                    nc.scalar.activation(fo_sb[:], fops[:], Act.Identity,
                                         bias=outb_s[:, 0:1], scale=1.0)
                    nc.sync.dma_start(out_d[:, t * 128 : (t + 1) * 128], fo_sb[:])

    nc.finalize()
    return nc


def _host_prep(inputs):
    """Prepare per-core input maps from full inputs."""
    feats = np.asarray(inputs["feats"], np.float32)          # [B, C, H, W]
    anchor = np.asarray(inputs["anchor_points"], np.float32)  # [B, HW, 2]
    value_w = np.asarray(inputs["value_w"], np.float32)
    value_b = np.asarray(inputs["value_b"], np.float32)
    weights_w = np.asarray(inputs["weights_w"], np.float32)
    weights_b = np.asarray(inputs["weights_b"], np.float32)
    offset_w = np.asarray(inputs["offset_w"], np.float32)
    offset_b = np.asarray(inputs["offset_b"], np.float32)
    out_w = np.asarray(inputs["out_w"], np.float32)
    out_b = np.asarray(inputs["out_b"], np.float32)

    shared = {
        "vwT16": np.ascontiguousarray(value_w.T).astype(ml_dtypes.bfloat16),
        "w72T": np.ascontiguousarray(weights_w.T).astype(ml_dtypes.bfloat16),
        "w18T": np.ascontiguousarray(offset_w.T),
        "owT16": np.ascontiguousarray(out_w.T).astype(ml_dtypes.bfloat16),
        "b72r": np.broadcast_to(weights_b, (128, 72)).copy(),
        "b18r": np.broadcast_to(offset_b, (128, 18)).copy(),
        "bvr": np.broadcast_to(value_b, (128, C)).copy(),
        "outb": out_b.reshape(128, 1).copy(),
        "ident16": np.eye(128, dtype=ml_dtypes.bfloat16),
    }
    oneh = np.zeros((128, 8, 128), np.float32)
    for qh in range(8):
        for m in range(128):
            oneh[16 * qh + (m % 16), qh, m] = 1.0
    shared["oneh"] = oneh.reshape(128, 8 * 128)

    in_maps = []
    feats16_b = [feats[b].reshape(C, HW).astype(ml_dtypes.bfloat16) for b in range(B)]
    for core in range(NCORES):
        b_i, sl = core // 4, core % 4
        off = sl * QS
        f16 = np.roll(feats16_b[b_i], -off, axis=1)
        f32r = np.roll(feats[b_i].reshape(C, HW), -off, axis=1)[:, :QS]
        an = anchor[b_i, off : off + QS].reshape(NT, 128, 2).transpose(1, 0, 2).reshape(128, NT * 2)
        m = dict(shared)
        m["feats16"] = np.ascontiguousarray(f16)
        m["feats32"] = np.ascontiguousarray(f32r)
        m["anch"] = np.ascontiguousarray(an)
        m["rotoff"] = np.full((128, 1), float(off), np.float32)
        in_maps.append(m)
    return in_maps


def kernel(**inputs) -> np.ndarray:
    from concourse.bass_utils import run_bass_kernel_spmd

    if "nc" not in _CACHE:
        _CACHE["nc"] = _build_nc()
    nc = _CACHE["nc"]
    in_maps = _host_prep(inputs)
    res = run_bass_kernel_spmd(nc, in_maps, core_ids=list(range(NCORES)))
    out = np.zeros((B, C, HW), np.float32)
    for core in range(NCORES):
        b_i, sl = core // 4, core % 4
        out[b_i, :, sl * QS : (sl + 1) * QS] = res.results[core]["out"]
    return out.reshape(B, C, H, W)


# revision 19
# speedup vs baseline: 1.1649x; 1.1649x over previous
"""Deformable 2D feature aggregator — Trainium2 Bass kernel, 8-core SPMD. v3.

Problem: B=2, C=128, H=96, W=160, P=9 points, G=8 groups.
  value = conv1x1(feats); w = softmax over P of conv1x1(feats); offs = conv1x1(feats)
  pts = anchors + offs; out_proj(conv-weighted bilinear gather of value at pts).

Sharding: 8 cores = 2 batches x 4 query-slices; each core builds the full
(rotated) value map for its batch, then gathers per-query corner windows.

v3 vs v2:
  - coords -> idxf -> idx16 chain emitted FIRST so the gather stream (gated on
    idx16 + valscr2) starts ~50us earlier; softmax/kw overlap under gathers.
  - last two gather chunks are single-tile so the final combine tail is half.
  - gather pool triple-buffered to absorb combine latency between chunks.
"""
import sys

sys.path.insert(0, "/opt/trn_rl_repo")

import numpy as np
import ml_dtypes

import concourse.bass as bass
import concourse.bacc as bacc
import concourse.mybir as mybir
import concourse.tile as tile
from concourse import library_config
from concourse.ap import AP

# problem constants (hardcoded per harness contract)
B, C, H, W = 2, 128, 96, 160
HW = H * W                     # 15360
P, G, GC = 9, 8, 16
NCORES = 8
QS = B * HW // NCORES          # 3840 queries per core
NT = QS // 128                 # 30 query tiles
TCH = 2                        # query tiles per gather chunk
NCH = NT // TCH                # 15 gather chunks
NP_ = NT * P                   # 270 points per partition-row
NIDX_CH = TCH * 128 * P        # 2304 gather indices per chunk
SHIFT = 1024.0                 # floor-bias (exact in f32 for our range)
NPXT = HW // 128               # 120 pixel tiles
VT = 12                        # pixel tiles per valscr2 write chunk
NCHV = NPXT // VT              # 24 write chunks

f32 = mybir.dt.float32
bf16 = mybir.dt.bfloat16
i16 = mybir.dt.int16
Alu = mybir.AluOpType
Act = mybir.ActivationFunctionType
Ax = mybir.AxisListType

_CACHE: dict = {}


def _build_nc():
    nc = bacc.Bacc()

    feats16 = nc.dram_tensor("feats16", [C, HW], bf16, kind="ExternalInput")
    feats32 = nc.dram_tensor("feats32", [C, QS], f32, kind="ExternalInput")
    anch = nc.dram_tensor("anch", [128, NT * 2], f32, kind="ExternalInput")
    vwT16 = nc.dram_tensor("vwT16", [C, C], bf16, kind="ExternalInput")
    w72T = nc.dram_tensor("w72T", [C, 72], bf16, kind="ExternalInput")
    w18T = nc.dram_tensor("w18T", [C, 18], f32, kind="ExternalInput")
    owT16 = nc.dram_tensor("owT16", [C, C], bf16, kind="ExternalInput")
    b72r = nc.dram_tensor("b72r", [128, 72], f32, kind="ExternalInput")
    b18r = nc.dram_tensor("b18r", [128, 18], f32, kind="ExternalInput")
    bvr = nc.dram_tensor("bvr", [128, C], f32, kind="ExternalInput")
    outb = nc.dram_tensor("outb", [128, 1], f32, kind="ExternalInput")
    oneh = nc.dram_tensor("oneh", [128, 8 * 128], f32, kind="ExternalInput")
    ident16 = nc.dram_tensor("ident16", [128, 128], bf16, kind="ExternalInput")
    rotoff = nc.dram_tensor("rotoff", [128, 1], f32, kind="ExternalInput")
    out_d = nc.dram_tensor("out", [C, QS], f32, kind="ExternalOutput")

    with tile.TileContext(nc) as tc, nc.allow_low_precision("bf16 combine by design"):
        with (
            tc.tile_pool(name="const", bufs=1) as cpool,
            tc.tile_pool(name="stage", bufs=1) as spool,
            tc.tile_pool(name="vsb", bufs=3) as vpool,
            tc.tile_pool(name="g", bufs=2) as gpool,
            tc.tile_pool(name="tprime", bufs=2) as tpool,
            tc.tile_pool(name="aggp", bufs=2) as apool,
            tc.tile_pool(name="ps", bufs=1, space="PSUM") as pspool,
            tc.tile_pool(name="dram", bufs=1, space="DRAM") as dpool,
        ):
            # ---- persistent loads (query-path tensors first: they gate idx16) ----
            f32s = spool.tile([128, QS], f32)
            nc.sync.dma_start(f32s[:], feats32[:])
            w72T_s = cpool.tile([C, 72], bf16)
            nc.sync.dma_start(w72T_s[:], w72T[:])
            w18T_s = cpool.tile([C, 18], f32)
            nc.sync.dma_start(w18T_s[:], w18T[:])
            anch_s = cpool.tile([128, NT * 2], f32)
            nc.sync.dma_start(anch_s[:], anch[:])
            rot_s = cpool.tile([128, 1], f32)
            nc.sync.dma_start(rot_s[:], rotoff[:])
            oneh_s = cpool.tile([128, 8 * 128], f32)
            nc.sync.dma_start(oneh_s[:], oneh[:])
            b72_s = cpool.tile([128, 72], f32)
            nc.sync.dma_start(b72_s[:], b72r[:])
            b18_s = cpool.tile([128, 18], f32)
            nc.sync.dma_start(b18_s[:], b18r[:])
            f16s = spool.tile([128, HW], bf16)
            nc.sync.dma_start(f16s[:], feats16[:])
            vwT_s = cpool.tile([C, C], bf16)
            nc.sync.dma_start(vwT_s[:], vwT16[:])
            owT_s = cpool.tile([C, C], bf16)
            nc.sync.dma_start(owT_s[:], owT16[:])
            bvr_s = cpool.tile([128, C], f32)
            nc.sync.dma_start(bvr_s[:], bvr[:])
            outb_s = cpool.tile([128, 1], f32)
            nc.sync.dma_start(outb_s[:], outb[:])
            ident_s = cpool.tile([128, 128], bf16)
            nc.sync.dma_start(ident_s[:], ident16[:])

            # dummy gather: preloads the GPSIMD gather-library IRAM during
            # the head so the first real gather doesn't pay the ~12us load.
            dumscr = dpool.tile([17, 2 * C], bf16)
            dumsrc = AP(tensor=dumscr.tensor, offset=dumscr[:, :].offset,
                        ap=[[2 * C, 16], [1, 4 * C]])
            dumidx = spool.tile([128, 1], i16)
            nc.vector.memset(dumidx[:], 0)
            dumout = spool.tile([128, 1, 4 * C], bf16)
            nc.gpsimd.dma_gather(
                dumout[:, :, :], dumsrc, dumidx[:, 0:1],
                num_idxs=16, num_idxs_reg=16,
                elem_size=4 * C, elem_step=2 * C, single_packet=False,
            )

            # pair-row scratch: record r = [V_rot(r), V_rot((r+W) mod HW)].
            # +1 pad record (= record 0) backs the r0+1 read at r0 = HW-1,
            # which is reachable after rotation.
            valscr2 = dpool.tile([HW + 1, 2 * C], bf16)

            proj72 = spool.tile([128, NT * 72], bf16)
            proj18 = spool.tile([128, NT * 18], f32)

            def tt(out, in0, in1, op):
                nc.vector.tensor_tensor(out=out, in0=in0, in1=in1, op=op)

            # ---- phase A0: query projections (offsets f32; wlog bf16) ----
            for t in range(NT):
                pp18 = pspool.tile([128, 18], f32, tag="pp", bufs=2,
                                   name=f"pp18_{t}")
                nc.tensor.matmul(pp18[:], f32s[:, t * 128 : (t + 1) * 128],
                                 w18T_s[:], start=True, stop=True)
                nc.vector.tensor_tensor(
                    out=proj18[:, t * 18 : (t + 1) * 18],
                    in0=pp18[:], in1=b18_s[:], op=Alu.add)
            for t in range(NT):
                pp72 = pspool.tile([128, 72], f32, tag="pp", bufs=2,
                                   name=f"pp72_{t}")
                nc.tensor.matmul(pp72[:], f16s[:, t * 128 : (t + 1) * 128],
                                 w72T_s[:], start=True, stop=True)
                nc.vector.tensor_tensor(
                    out=proj72[:, t * 72 : (t + 1) * 72],
                    in0=pp72[:], in1=b72_s[:], op=Alu.add)

            # ---- phase B0: coords -> gather record indices (critical path) ----
            px = spool.tile([128, NP_], f32, tag="px")
            py = spool.tile([128, NP_], f32, tag="py")
            offs_x = AP(tensor=proj18.tensor, offset=proj18[:, :].offset,
                        ap=[[proj18[:, :].ap[0][0], 128], [18, NT], [2, P]])
            offs_y = AP(tensor=proj18.tensor, offset=proj18[:, :].offset + 1,
                        ap=[[proj18[:, :].ap[0][0], 128], [18, NT], [2, P]])
            anx = AP(tensor=anch_s.tensor, offset=anch_s[:, :].offset,
                     ap=[[anch_s[:, :].ap[0][0], 128], [2, NT], [0, P]])
            any_ = AP(tensor=anch_s.tensor, offset=anch_s[:, :].offset + 1,
                      ap=[[anch_s[:, :].ap[0][0], 128], [2, NT], [0, P]])
            tt(px[:, :].rearrange("p (t q) -> p t q", q=P), offs_x, anx, Alu.add)
            tt(py[:, :].rearrange("p (t q) -> p t q", q=P), offs_y, any_, Alu.add)

            xp = spool.tile([128, NP_], f32)
            yp = spool.tile([128, NP_], f32)
            nc.scalar.activation(xp[:], px[:], Act.Copy, bias=SHIFT - 0.5, scale=float(W))
            nc.scalar.activation(yp[:], py[:], Act.Copy, bias=SHIFT - 0.5, scale=float(H))
            # floor via round(x-0.5): (x + (2^23-0.5)) - 2^23. At integer x the
            # half-even tie may floor one low with frac 1.0 — an equivalent
            # bilinear weighting, so interpolation is unchanged.
            MAGIC = float(1 << 23)
            xf = spool.tile([128, NP_], f32, tag="px")   # reuse px slot
            yf = spool.tile([128, NP_], f32, tag="py")   # reuse py slot
            nc.vector.tensor_scalar(out=xf[:], in0=xp[:], scalar1=MAGIC - 0.5,
                                    scalar2=MAGIC, op0=Alu.add, op1=Alu.subtract)
            nc.vector.tensor_scalar(out=yf[:], in0=yp[:], scalar1=MAGIC - 0.5,
                                    scalar2=MAGIC, op0=Alu.add, op1=Alu.subtract)

            xg = spool.tile([128, NP_], f32)
            nc.vector.tensor_scalar(out=xg[:], in0=xf[:], scalar1=SHIFT, scalar2=0.0,
                                    op0=Alu.subtract, op1=Alu.max)
            nc.vector.tensor_scalar(out=xg[:], in0=xg[:], scalar1=float(W - 2),
                                    scalar2=None, op0=Alu.min)
            yg = spool.tile([128, NP_], f32)
            nc.vector.tensor_scalar(out=yg[:], in0=yf[:], scalar1=SHIFT, scalar2=0.0,
                                    op0=Alu.subtract, op1=Alu.max)
            nc.vector.tensor_scalar(out=yg[:], in0=yg[:], scalar1=float(H - 2),
                                    scalar2=None, op0=Alu.min)

            # idx = (yg*W + xg - rotoff) mod HW
            v1 = spool.tile([128, NP_], f32)
            v2 = spool.tile([128, NP_], f32)
            idxf = spool.tile([128, NP_], f32)
            nc.scalar.activation(v1[:], yg[:], Act.Copy, bias=0.0, scale=float(W))
            tt(idxf[:], v1[:], xg[:], Alu.add)
            nc.vector.tensor_scalar(out=idxf[:], in0=idxf[:], scalar1=rot_s[:, 0:1],
                                    scalar2=None, op0=Alu.subtract)
            nc.vector.tensor_scalar(out=v2[:], in0=idxf[:], scalar1=0.0,
                                    scalar2=float(HW), op0=Alu.is_lt, op1=Alu.mult)
            tt(idxf[:], idxf[:], v2[:], Alu.add)

            # ---- phase A1: bf16 value map over the whole rotated image ----
            def emit_value_chunk(v):
                vsb5 = vpool.tile([128, VT * C], bf16, tag="vsb5", bufs=2,
                                  name=f"vsb5_{v}")
                for k4 in range(VT // 4):
                    t = v * VT + 4 * k4
                    vps = pspool.tile([128, 4 * C], f32, tag="vps", bufs=3,
                                      name=f"vps{t}")
                    for j in range(4):
                        nc.tensor.matmul(
                            vps[:, j * C : (j + 1) * C],
                            f16s[:, (t + j) * 128 : (t + j + 1) * 128],
                            vwT_s[:], start=True, stop=True)
                    nc.scalar.copy(vsb5[:, 4 * k4 * C : (4 * k4 + 4) * C], vps[:])
                base = v * VT * 128  # first pixel (row) of this chunk
                # first half: rows [base, base+640), cols 0:C
                dst1 = AP(tensor=valscr2.tensor,
                          offset=valscr2[:, :].offset + base * 2 * C,
                          ap=[[2 * C, 128], [128 * 2 * C, VT], [1, C]])
                src1 = AP(tensor=vsb5.tensor, offset=vsb5[:, :].offset,
                          ap=[[vsb5[:, :].ap[0][0], 128], [C, VT], [1, C]])
                nc.sync.dma_start(dst1, src1)
                # second half: rows [(base - W) mod HW ...), cols C:2C
                lo = base - W
                if lo >= 0:
                    dst2 = AP(tensor=valscr2.tensor,
                              offset=valscr2[:, :].offset + lo * 2 * C + C,
                              ap=[[2 * C, 128], [128 * 2 * C, VT], [1, C]])
                    nc.sync.dma_start(dst2, src1)
                else:
                    # v == 0: rows [HW-160, HW) from (b=0 all p) + (b=1 p<32),
                    # then rows [0, 96) from (b=1 p>=32), rows [96, 480) b=2..4
                    d_a = AP(tensor=valscr2.tensor,
                             offset=valscr2[:, :].offset + (HW - W) * 2 * C + C,
                             ap=[[2 * C, 128], [1, C]])
                    s_a = AP(tensor=vsb5.tensor, offset=vsb5[:, :].offset,
                             ap=[[vsb5[:, :].ap[0][0], 128], [1, C]])
                    nc.sync.dma_start(d_a, s_a)
                    d_b = AP(tensor=valscr2.tensor,
                             offset=valscr2[:, :].offset + (HW - 32) * 2 * C + C,
                             ap=[[2 * C, 32], [1, C]])
                    s_b = AP(tensor=vsb5.tensor, offset=vsb5[:, :].offset + C,
                             ap=[[vsb5[:, :].ap[0][0], 32], [1, C]])
                    nc.sync.dma_start(d_b, s_b)
                    d_c = AP(tensor=valscr2.tensor,
                             offset=valscr2[:, :].offset + 0 * 2 * C + C,
                             ap=[[2 * C, 96], [1, C]])
                    s_c = AP(tensor=vsb5.tensor,
                             offset=vsb5[32:, :].offset + C,
                             ap=[[vsb5[:, :].ap[0][0], 96], [1, C]])
                    nc.sync.dma_start(d_c, s_c)
                    d_d = AP(tensor=valscr2.tensor,
                             offset=valscr2[:, :].offset + 96 * 2 * C + C,
                             ap=[[2 * C, 128], [128 * 2 * C, VT - 2], [1, C]])
                    s_d = AP(tensor=vsb5.tensor,
                             offset=vsb5[:, :].offset + 2 * C,
                             ap=[[vsb5[:, :].ap[0][0], 128], [C, VT - 2], [1, C]])
                    nc.sync.dma_start(d_d, s_d)
                    # pad record HW = record 0 = [V_rot(0), V_rot(W)]
                    d_p0 = AP(tensor=valscr2.tensor,
                              offset=valscr2[:, :].offset + HW * 2 * C,
                              ap=[[2 * C, 1], [1, C]])
                    s_p0 = AP(tensor=vsb5.tensor, offset=vsb5[:, :].offset,
                              ap=[[vsb5[:, :].ap[0][0], 1], [1, C]])
                    nc.sync.dma_start(d_p0, s_p0)
                    d_p1 = AP(tensor=valscr2.tensor,
                              offset=valscr2[:, :].offset + HW * 2 * C + C,
                              ap=[[2 * C, 1], [1, C]])
                    s_p1 = AP(tensor=vsb5.tensor,
                              offset=vsb5[32:, :].offset + C,
                              ap=[[vsb5[:, :].ap[0][0], 1], [1, C]])
                    nc.sync.dma_start(d_p1, s_p1)


            for v in range(3):
                emit_value_chunk(v)

            # chunk schedule: 14 double-tile chunks + 2 single-tile chunks
            # (small tail chunks shorten the final gather-drain + combine).
            CHUNKS = [(2 * i, 2) for i in range(14)] + [(28, 1), (29, 1)]
            COLOFF = []
            acc = 0
            for (_, ntile) in CHUNKS:
                COLOFF.append(acc)
                acc += ntile * P * 8

            # idx16: position i in chunk = jj*128 + q (jj = tt*P + pt),
            # stored at idx16[q%16 (+16a), coloff + 8*jj + q//16]
            idx16 = spool.tile([128, NT * P * 8], i16)
            for qh in range(8):
                i16ps = pspool.tile([128, NP_], f32, tag="pp", bufs=2,
                                    name=f"i16ps{qh}")
                nc.tensor.matmul(i16ps[:], oneh_s[:, qh * 128 : (qh + 1) * 128],
                                 idxf[:], start=True, stop=True)
                dst = AP(tensor=idx16.tensor, offset=idx16[:, :].offset + qh,
                         ap=[[idx16[:, :].ap[0][0], 128],
                             [2 * P * 8, 14], [P * 8, 2], [8, P]])
                src = AP(tensor=i16ps.tensor, offset=i16ps[:, :].offset,
                         ap=[[i16ps[:, :].ap[0][0], 128],
                             [2 * P, 14], [P, 2], [1, P]])
                nc.vector.tensor_copy(dst, src)
                # tail tiles 28, 29 (single-tile chunks, contiguous blocks)
                dst_t = AP(tensor=idx16.tensor,
                           offset=idx16[:, :].offset + qh + 28 * P * 8,
                           ap=[[idx16[:, :].ap[0][0], 128], [P * 8, 2], [8, P]])
                src_t = AP(tensor=i16ps.tensor,
                           offset=i16ps[:, :].offset + 28 * P,
                           ap=[[i16ps[:, :].ap[0][0], 128], [P, 2], [1, P]])
                nc.vector.tensor_copy(dst_t, src_t)

            for v in range(3, NCHV):
                emit_value_chunk(v)

            # ---- phase B1: softmax / bilinear weights (overlaps gathers) ----
            wx = spool.tile([128, NP_], f32)
            wy = spool.tile([128, NP_], f32)
            tt(wx[:], xp[:], xf[:], Alu.subtract)
            tt(wy[:], yp[:], yf[:], Alu.subtract)
            ux = spool.tile([128, NP_], f32)
            uy = spool.tile([128, NP_], f32)
            nc.scalar.activation(ux[:], wx[:], Act.Copy, bias=1.0, scale=-1.0)
            nc.scalar.activation(uy[:], wy[:], Act.Copy, bias=1.0, scale=-1.0)

            # softmax over points
            wmax = spool.tile([128, NT * G], f32, tag="wmax")
            wl_gp = AP(tensor=proj72.tensor, offset=proj72[:, :].offset,
                       ap=[[proj72[:, :].ap[0][0], 128], [72, NT], [1, G], [G, P]])
            nc.vector.tensor_reduce(out=wmax[:, :].rearrange("p (t g) -> p t g", g=G),
                                    in_=wl_gp, axis=Ax.X, op=Alu.max)
            smf = spool.tile([128, NT * P * G], f32, tag="smf")
            wl_pg = AP(tensor=proj72.tensor, offset=proj72[:, :].offset,
                       ap=[[proj72[:, :].ap[0][0], 128], [72, NT], [G, P], [1, G]])
            wmax_b = AP(tensor=wmax.tensor, offset=wmax[:, :].offset,
                        ap=[[wmax[:, :].ap[0][0], 128], [G, NT], [0, P], [1, G]])
            tt(smf[:, :].rearrange("p (t q g) -> p t q g", q=P, g=G),
               wl_pg, wmax_b, Alu.subtract)
            nc.scalar.activation(smf[:], smf[:], Act.Exp)
            ssum = spool.tile([128, NT * G], f32, tag="wmax")
            sm_gp = AP(tensor=smf.tensor, offset=smf[:, :].offset,
                       ap=[[smf[:, :].ap[0][0], 128], [P * G, NT], [1, G], [G, P]])
            nc.vector.tensor_reduce(out=ssum[:, :].rearrange("p (t g) -> p t g", g=G),
                                    in_=sm_gp, axis=Ax.X, op=Alu.add)
            rcps = spool.tile([128, NT * G], f32)
            nc.vector.reciprocal(rcps[:], ssum[:])
            wsm = spool.tile([128, NT * P * G], bf16)
            rcp_b = AP(tensor=rcps.tensor, offset=rcps[:, :].offset,
                       ap=[[rcps[:, :].ap[0][0], 128], [G, NT], [0, P], [1, G]])
            tt(wsm[:, :].rearrange("p (t q g) -> p t q g", q=P, g=G),
               smf[:, :].rearrange("p (t q g) -> p t q g", q=P, g=G),
               rcp_b, Alu.mult)

            # x-validity masks (with pair-clamp weight swap)
            tA = spool.tile([128, NP_], f32)
            tB = spool.tile([128, NP_], f32)
            mAx = spool.tile([128, NP_], f32)
            nc.vector.tensor_scalar(out=tA[:], in0=xf[:], scalar1=SHIFT,
                                    scalar2=None, op0=Alu.is_ge)
            nc.vector.tensor_scalar(out=tB[:], in0=xf[:], scalar1=SHIFT + W - 2,
                                    scalar2=None, op0=Alu.is_le)
            tt(mAx[:], tA[:], tB[:], Alu.mult)
            mBx = spool.tile([128, NP_], f32)
            nc.vector.tensor_scalar(out=mBx[:], in0=xf[:], scalar1=SHIFT - 1.0,
                                    scalar2=None, op0=Alu.is_equal)
            mCx = spool.tile([128, NP_], f32)
            nc.vector.tensor_scalar(out=mCx[:], in0=xf[:], scalar1=SHIFT + W - 1,
                                    scalar2=None, op0=Alu.is_equal)

            bx = spool.tile([128, NT * P * 2], f32)   # (t, pt, side)
            tt(v1[:], ux[:], mAx[:], Alu.mult)
            tt(v2[:], wx[:], mBx[:], Alu.mult)
            bx0 = AP(tensor=bx.tensor, offset=bx[:, :].offset,
                     ap=[[bx[:, :].ap[0][0], 128], [2, NP_]])
            tt(bx0, v1[:], v2[:], Alu.add)
            tt(v1[:], wx[:], mAx[:], Alu.mult)
            tt(v2[:], ux[:], mCx[:], Alu.mult)
            bx1 = AP(tensor=bx.tensor, offset=bx[:, :].offset + 1,
                     ap=[[bx[:, :].ap[0][0], 128], [2, NP_]])
            tt(bx1, v1[:], v2[:], Alu.add)

            # y masks (swap form, mirrors x): slot0 = row yg, slot1 = row yg+1
            my = spool.tile([128, NP_], f32)
            nc.vector.tensor_scalar(out=tA[:], in0=yf[:], scalar1=SHIFT,
                                    scalar2=None, op0=Alu.is_ge)
            nc.vector.tensor_scalar(out=tB[:], in0=yf[:], scalar1=SHIFT + H - 2,
                                    scalar2=None, op0=Alu.is_le)
            tt(my[:], tA[:], tB[:], Alu.mult)
            myB = spool.tile([128, NP_], f32)
            nc.vector.tensor_scalar(out=myB[:], in0=yf[:], scalar1=SHIFT - 1.0,
                                    scalar2=None, op0=Alu.is_equal)
            myC = spool.tile([128, NP_], f32)
            nc.vector.tensor_scalar(out=myC[:], in0=yf[:], scalar1=SHIFT + H - 1,
                                    scalar2=None, op0=Alu.is_equal)
            by = spool.tile([128, NT * P * 2], f32)   # (t, pt, yy)
            by0 = AP(tensor=by.tensor, offset=by[:, :].offset,
                     ap=[[by[:, :].ap[0][0], 128], [2, NP_]])
            by1 = AP(tensor=by.tensor, offset=by[:, :].offset + 1,
                     ap=[[by[:, :].ap[0][0], 128], [2, NP_]])
            tt(v1[:], uy[:], my[:], Alu.mult)
            tt(v2[:], wy[:], myB[:], Alu.mult)
            tt(by0, v1[:], v2[:], Alu.add)
            tt(v1[:], wy[:], my[:], Alu.mult)
            tt(v2[:], uy[:], myC[:], Alu.mult)
            tt(by1, v1[:], v2[:], Alu.add)

            # cw[t, pt, x, yy] = bx[t,pt,x] * by[t,pt,yy]
            cw = spool.tile([128, NT * P * 4], bf16)
            for x in range(2):
                by_v = AP(tensor=by.tensor, offset=by[:, :].offset,
                          ap=[[by[:, :].ap[0][0], 128], [2 * P, NT], [2, P], [1, 2]])
                bx_x = AP(tensor=bx.tensor, offset=bx[:, :].offset + x,
                          ap=[[bx[:, :].ap[0][0], 128], [2 * P, NT], [2, P], [0, 2]])
                cw_x = AP(tensor=cw.tensor, offset=cw[:, :].offset + 2 * x,
                          ap=[[cw[:, :].ap[0][0], 128], [4 * P, NT], [4, P], [1, 2]])
                tt(cw_x, by_v, bx_x, Alu.mult)

            # kw[t, pt, rs, g] = cw[t, pt, rs] * wsm[t, pt, g]  (bf16)
            kw = spool.tile([128, NT * P * 4 * G], bf16)
            for rs in range(4):
                cw_rs = AP(tensor=cw.tensor, offset=cw[:, :].offset + rs,
                           ap=[[cw[:, :].ap[0][0], 128], [4 * P, NT], [4, P], [0, G]])
                w_v = AP(tensor=wsm.tensor, offset=wsm[:, :].offset,
                         ap=[[wsm[:, :].ap[0][0], 128], [P * G, NT], [G, P], [1, G]])
                kw_rs = AP(tensor=kw.tensor, offset=kw[:, :].offset + rs * G,
                           ap=[[kw[:, :].ap[0][0], 128], [4 * P * G, NT], [4 * G, P], [1, G]])
                tt(kw_rs, cw_rs, w_v, Alu.mult)

            # sumcoef[t, g] = sum_pt wsm * (bx0+bx1)*(by0+by1)   (for value_b)
            bx0r = AP(tensor=bx.tensor, offset=bx[:, :].offset,
                      ap=[[bx[:, :].ap[0][0], 128], [2, NP_]])
            bx1r = AP(tensor=bx.tensor, offset=bx[:, :].offset + 1,
                      ap=[[bx[:, :].ap[0][0], 128], [2, NP_]])
            by0r = AP(tensor=by.tensor, offset=by[:, :].offset,
                      ap=[[by[:, :].ap[0][0], 128], [2, NP_]])
            by1r = AP(tensor=by.tensor, offset=by[:, :].offset + 1,
                      ap=[[by[:, :].ap[0][0], 128], [2, NP_]])
            tt(v1[:], bx0r, bx1r, Alu.add)
            tt(v2[:], by0r, by1r, Alu.add)
            bws = spool.tile([128, NP_], bf16)
            tt(bws[:], v1[:], v2[:], Alu.mult)
            wp = spool.tile([128, NT * P * G], bf16, tag="smf")  # reuse smf slot
            bws_b = AP(tensor=bws.tensor, offset=bws[:, :].offset,
                       ap=[[bws[:, :].ap[0][0], 128], [P, NT], [1, P], [0, G]])
            tt(wp[:, :].rearrange("p (t q g) -> p t q g", q=P, g=G),
               wsm[:, :].rearrange("p (t q g) -> p t q g", q=P, g=G),
               bws_b, Alu.mult)
            sumcoef = spool.tile([128, NT * G], f32)
            wp_gp = AP(tensor=wp.tensor, offset=wp[:, :].offset,
                       ap=[[wp[:, :].ap[0][0], 128], [P * G, NT], [1, G], [G, P]])
            nc.vector.tensor_reduce(out=sumcoef[:, :].rearrange("p (t g) -> p t g", g=G),
                                    in_=wp_gp, axis=Ax.X, op=Alu.add)

            # ---- phase C: gather chunks + combine ----
            val_src = AP(tensor=valscr2.tensor, offset=valscr2[:, :].offset,
                         ap=[[2 * C, HW], [1, 4 * C]])

            for ch, (t0, ntile) in enumerate(CHUNKS):
                nidx = ntile * P * 128
                gt = gpool.tile([128, TCH * P, 4 * C], bf16, tag="g", bufs=3,
                                name=f"g{ch}")
                last = ch == len(CHUNKS) - 1
                if not last:
                    nc.gpsimd.dma_gather(
                        gt[:, : ntile * P, :], val_src,
                        idx16[:, COLOFF[ch] : COLOFF[ch] + nidx // 16],
                        num_idxs=nidx, num_idxs_reg=nidx,
                        elem_size=4 * C, elem_step=2 * C, single_packet=False,
                    )
                else:
                    # split the final gather so the tail DMA drain is short
                    nA = 5 * 128
                    nc.gpsimd.dma_gather(
                        gt[:, :5, :], val_src,
                        idx16[:, COLOFF[ch] : COLOFF[ch] + nA // 16],
                        num_idxs=nA, num_idxs_reg=nA,
                        elem_size=4 * C, elem_step=2 * C, single_packet=False,
                    )
                    nB = 4 * 128
                    nc.gpsimd.dma_gather(
                        gt[:, 5:9, :], val_src,
                        idx16[:, COLOFF[ch] + nA // 16 : COLOFF[ch] + P * 8],
                        num_idxs=nB, num_idxs_reg=nB,
                        elem_size=4 * C, elem_step=2 * C, single_packet=False,
                    )

                for tt_ in range(ntile):
                    t = t0 + tt_
                    # tp[q, (j36, c)] = gt * kw  (j36 = (pt, x, yy), c = (g, gc))
                    tp = tpool.tile([128, P * 4 * C], bf16, tag="tp", bufs=1,
                                    name=f"tp{t}")
                    g_v = AP(tensor=gt.tensor,
                             offset=gt[:, :, :].offset + tt_ * P * 4 * C,
                             ap=[[gt[:, :, :].ap[0][0], 128],
                                 [C, P * 4], [GC, G], [1, GC]])
                    kw_v = AP(tensor=kw.tensor, offset=kw[:, :].offset + t * P * 4 * G,
                              ap=[[kw[:, :].ap[0][0], 128],
                                  [G, P * 4], [1, G], [0, GC]])
                    tp_v = AP(tensor=tp.tensor, offset=tp[:, :].offset,
                              ap=[[tp[:, :].ap[0][0], 128],
                                  [C, P * 4], [GC, G], [1, GC]])
                    nc.vector.tensor_tensor(out=tp_v, in0=g_v, in1=kw_v, op=Alu.mult)

                    # pre-add yy pairs (2x mode): tp2[q, ((pt,x), c)]
                    tp2 = tpool.tile([128, P * 2 * C], bf16, tag="tp2", name=f"tp2{t}")
                    in0 = AP(tensor=tp.tensor, offset=tp[:, :].offset,
                             ap=[[tp[:, :].ap[0][0], 128], [2 * C, 2 * P], [1, C]])
                    in1 = AP(tensor=tp.tensor, offset=tp[:, :].offset + C,
                             ap=[[tp[:, :].ap[0][0], 128], [2 * C, 2 * P], [1, C]])
                    out2 = AP(tensor=tp2.tensor, offset=tp2[:, :].offset,
                              ap=[[tp2[:, :].ap[0][0], 128], [C, 2 * P], [1, C]])
                    nc.vector.tensor_tensor(out=out2, in0=in0, in1=in1, op=Alu.add)

                    # ebias[q, c] = value_b[c] * sumcoef[q, g(c)]  (bf16)
                    ebias = apool.tile([128, C], bf16, tag="eb", bufs=1, name=f"eb{t}")
                    sc_v = AP(tensor=sumcoef.tensor,
                              offset=sumcoef[:, :].offset + t * G,
                              ap=[[sumcoef[:, :].ap[0][0], 128], [1, G], [0, GC]])
                    bv_v = bvr_s[:, :].rearrange("p (g c) -> p g c", g=G)
                    nc.vector.tensor_tensor(
                        out=ebias[:, :].rearrange("p (g c) -> p g c", g=G),
                        in0=sc_v, in1=bv_v, op=Alu.mult)

                    # 19 transpose-accumulate matmuls: aggT[c, q] in PSUM
                    aggT = pspool.tile([128, 128], f32, tag="mm128", bufs=2,
                                       name=f"aggT{t}")
                    for k in range(2 * P):
                        nc.tensor.matmul(aggT[:], tp2[:, k * C : (k + 1) * C],
                                         ident_s[:], start=(k == 0), stop=False)
                    nc.tensor.matmul(aggT[:], ebias[:], ident_s[:],
                                     start=False, stop=True)

                    # out-projection straight off the accumulated PSUM
                    aT = apool.tile([128, 128], bf16, tag="aT", bufs=1, name=f"aT{t}")
                    nc.scalar.copy(aT[:], aggT[:])
                    fops = pspool.tile([128, C], f32, tag="mm128", bufs=2,
                                       name=f"fo{t}")
                    nc.tensor.matmul(fops[:], owT_s[:], aT[:], start=True, stop=True)
                    fo_sb = apool.tile([128, C], f32, tag="fosb", name=f"fosb{t}")
                    nc.scalar.activation(fo_sb[:], fops[:], Act.Identity,
                                         bias=outb_s[:, 0:1], scale=1.0)
                    nc.sync.dma_start(out_d[:, t * 128 : (t + 1) * 128], fo_sb[:])

    nc.finalize()
    return nc


def _host_prep(inputs):
    """Prepare per-core input maps from full inputs."""
    feats = np.asarray(inputs["feats"], np.float32)          # [B, C, H, W]
    anchor = np.asarray(inputs["anchor_points"], np.float32)  # [B, HW, 2]
    value_w = np.asarray(inputs["value_w"], np.float32)
    value_b = np.asarray(inputs["value_b"], np.float32)
    weights_w = np.asarray(inputs["weights_w"], np.float32)
    weights_b = np.asarray(inputs["weights_b"], np.float32)
    offset_w = np.asarray(inputs["offset_w"], np.float32)
    offset_b = np.asarray(inputs["offset_b"], np.float32)
    out_w = np.asarray(inputs["out_w"], np.float32)
    out_b = np.asarray(inputs["out_b"], np.float32)

    shared = {
        "vwT16": np.ascontiguousarray(value_w.T).astype(ml_dtypes.bfloat16),
        "w72T": np.ascontiguousarray(weights_w.T).astype(ml_dtypes.bfloat16),
        "w18T": np.ascontiguousarray(offset_w.T),
        "owT16": np.ascontiguousarray(out_w.T).astype(ml_dtypes.bfloat16),
        "b72r": np.broadcast_to(weights_b, (128, 72)).copy(),
        "b18r": np.broadcast_to(offset_b, (128, 18)).copy(),
        "bvr": np.broadcast_to(value_b, (128, C)).copy(),
        "outb": out_b.reshape(128, 1).copy(),
        "ident16": np.eye(128, dtype=ml_dtypes.bfloat16),
    }
    oneh = np.zeros((128, 8, 128), np.float32)
    for qh in range(8):
        for m in range(128):
            oneh[16 * qh + (m % 16), qh, m] = 1.0
    shared["oneh"] = oneh.reshape(128, 8 * 128)

    in_maps = []
    feats16_b = [feats[b].reshape(C, HW).astype(ml_dtypes.bfloat16) for b in range(B)]
    for core in range(NCORES):
        b_i, sl = core // 4, core % 4
        off = sl * QS
        f16 = np.roll(feats16_b[b_i], -off, axis=1)
        f32r = np.roll(feats[b_i].reshape(C, HW), -off, axis=1)[:, :QS]
        an = anchor[b_i, off : off + QS].reshape(NT, 128, 2).transpose(1, 0, 2).reshape(128, NT * 2)
        m = dict(shared)
        m["feats16"] = np.ascontiguousarray(f16)
        m["feats32"] = np.ascontiguousarray(f32r)
        m["anch"] = np.ascontiguousarray(an)
        m["rotoff"] = np.full((128, 1), float(off), np.float32)
        in_maps.append(m)
    return in_maps


def kernel(**inputs) -> np.ndarray:
    from concourse.bass_utils import run_bass_kernel_spmd

    if "nc" not in _CACHE:
        _CACHE["nc"] = _build_nc()
    nc = _CACHE["nc"]
    in_maps = _host_prep(inputs)
    res = run_bass_kernel_spmd(nc, in_maps, core_ids=list(range(NCORES)))
    out = np.zeros((B, C, HW), np.float32)
    for core in range(NCORES):
        b_i, sl = core // 4, core % 4
        out[b_i, :, sl * QS : (sl + 1) * QS] = res.results[core]["out"]
    return out.reshape(B, C, H, W)
